# revision 2
# baseline (speedup 1.0000x reference)
"""Trainium2 Bass kernel for a full attention layer (QKV proj + interleaved
RoPE + non-causal SDPA + output proj), tensor-parallel over heads on 8
NeuronCores.

Hardcoded problem shape: B=2, S=2048, HID=2048, H=16 heads, DH=128, fp32.

Sharding (per core c of 8): heads 2c, 2c+1.
 - w_qkv rows for those heads (q/k rows de-interleaved per head so RoPE's
   (2i, 2i+1) pairing becomes a 64-partition block swap), transposed to
   [HID, 256] so the contraction dim (HID) rides the SBUF partition axis.
 - w_o columns for those heads, transposed to [256, HID].
 - hidden_states transposed to [HID, B*S] (replicated to every core).
 - cos/sin prepped as de-interleaved, transposed [128, S] tiles; sin carries
   the rotate-half sign in its first 64 rows.
Each core computes a full-shape partial output [B*S, HID] (its heads'
contribution through w_o); the host unshards by summing the 8 partials.

All matmuls run as float32r (full PE rate for moving dim >= 256; fp32 data).
Attention is computed in the S^T orientation: scores come out as
P^T[k, q] tiles so the AV matmul can contract k on the partition axis with
no transposes anywhere.  The softmax denominator is an all-ones [128,128]
stationary matmul, which lands sum_k P broadcast across all 128 partitions
for free; out tiles are scaled by its reciprocal after AV (divide-after-AV).
exp() is fused into the PSUM->SBUF drain on the scalar engine with the
1/sqrt(DH) scale folded in.  No max-subtraction: scores are ~N(0,1) so
exp is safe in fp32.
"""

import os

import numpy as np

B, S, HID = 2, 2048, 2048
H, DH = 16, 128
NC = 8
HPC = H // NC          # heads per core = 2
OC = HPC * DH          # per-core o width per section = 256
T = B * S              # 4096 tokens
KT = HID // 128        # 16 contraction tiles
TC = 256               # token chunk for QKV projection
QC = 512               # query chunk for attention
SCALE = 1.0 / float(np.sqrt(DH))

_exec_time_ns = None   # stashed by kernel() for the test harness


def _build(reps=1):
    import concourse.bacc as bacc
    import concourse.mybir as mybir
    import concourse.tile as tile

    f32 = mybir.dt.float32
    fr = mybir.dt.float32r
    Exp = mybir.ActivationFunctionType.Exp

    def r(ap):
        return ap

    nc = bacc.Bacc("TRN2", target_bir_lowering=False)

    hT = nc.dram_tensor("hT", [HID, T], fr, kind="ExternalInput")
    wqT = nc.dram_tensor("wqT", [HID, OC], fr, kind="ExternalInput")
    wkT = nc.dram_tensor("wkT", [HID, OC], fr, kind="ExternalInput")
    wvT = nc.dram_tensor("wvT", [HID, OC], fr, kind="ExternalInput")
    woT = nc.dram_tensor("woT", [OC, HID], fr, kind="ExternalInput")
    cc = nc.dram_tensor("cc", [DH, S], f32, kind="ExternalInput")
    ss = nc.dram_tensor("ss", [DH, S], f32, kind="ExternalInput")
    out_p = nc.dram_tensor("out_p", [T, HID], f32, kind="ExternalOutput")

    hT_r = hT.rearrange("(k p) t -> p k t", p=128)      # [128, 16, T]

    with tile.TileContext(nc) as tc:
        with (
            tc.tile_pool(name="const", bufs=1) as constp,
            tc.tile_pool(name="hbuf", bufs=2) as hpool,
            tc.tile_pool(name="qkv", bufs=1) as qkvp,
            tc.tile_pool(name="rope", bufs=2) as ropep,
            tc.tile_pool(name="pbuf", bufs=6) as pp,
            tc.tile_pool(name="small", bufs=2) as smallp,
            tc.tile_pool(name="fout", bufs=4) as foutp,
        ):
            # ---- resident weights/constants (per-ktile tiles: 1 DMA -> 1 sem) ----
            # (re-emitted per rep for benchmarking; tags make slots reuse)
            for _rep in range(reps):
             wqT_r = wqT.rearrange("(k p) o -> p k o", p=128)
             wkT_r = wkT.rearrange("(k p) o -> p k o", p=128)
             wvT_r = wvT.rearrange("(k p) o -> p k o", p=128)
             woT_r = woT.rearrange("(h p) n -> p h n", p=128)
             wq_t, wk_t, wv_t = [], [], []
             for kk in range(KT):
                 for lst, srcr, nm in (
                     (wq_t, wqT_r, "wq"),
                     (wk_t, wkT_r, "wk"),
                     (wv_t, wvT_r, "wv"),
                 ):
                     t = constp.tile([128, OC], fr, tag=f"{nm}{kk}")
                     nc.sync.dma_start(out=t, in_=srcr[:, kk, :])
                     lst.append(t)
             wo_t = []
             for hl in range(HPC):
                 t = constp.tile([128, HID], fr, tag=f"wo{hl}")
                 nc.sync.dma_start(out=t, in_=woT_r[:, hl, :])
                 wo_t.append(t)
             cc_sb = constp.tile([128, S], f32)
             ss_sb = constp.tile([128, S], f32)
             nc.sync.dma_start(out=cc_sb, in_=cc[:, :])
             nc.sync.dma_start(out=ss_sb, in_=ss[:, :])
             ones_f32 = constp.tile([128, 128], f32)
             nc.vector.memset(ones_f32, 1.0)
             ones_sb = constp.tile([128, 128], fr)
             nc.scalar.copy(ones_sb, ones_f32)

             for b in range(B):
                 t0 = b * S

                 # ---- phase 1: QKV projection (+ fused RoPE for q,k) ----
                 # qk_sb rows: [q_h0, q_h1, k_h0, k_h1], each [128 d, S]
                 qk_sb = qkvp.tile([128, 4, S], fr, tag="qk")
                 v_sb = qkvp.tile([128, S // 128, OC], fr, tag="v")
                 w_of = [(wq_t, 0), (wq_t, 1), (wk_t, 0), (wk_t, 1)]
                 with tc.tile_pool(name="ps1", bufs=2, space="PSUM") as ps1:
                     for tci in range(S // TC):
                         soff = tci * TC
                         hch = []
                         for kk in range(KT):
                             ht = hpool.tile([128, TC], fr, tag=f"hch{kk}")
                             nc.sync.dma_start(
                                 out=ht, in_=hT_r[:, kk, t0 + soff : t0 + soff + TC]
                             )
                             hch.append(ht)
                         for ot in range(4):
                             wsb, hl = w_of[ot]
                             ps = ps1.tile([128, TC], f32, tag="ps_qk")
                             for kk in range(KT):
                                 nc.tensor.matmul(
                                     ps,
                                     r(wsb[kk][:, hl * DH : (hl + 1) * DH]),
                                     r(hch[kk]),
                                     start=(kk == 0),
                                     stop=(kk == KT - 1),
                                 )
                             # RoPE: dst = raw*cc + blockswap(raw)*ss_signed
                             raw = ropep.tile([128, TC], f32, tag="raw")
                             nc.scalar.copy(raw, ps)
                             swp = ropep.tile([128, TC], f32, tag="swp")
                             nc.sync.dma_start(out=swp[0:64, :], in_=raw[64:128, :])
                             nc.sync.dma_start(out=swp[64:128, :], in_=raw[0:64, :])
                             tmp = ropep.tile([128, TC], f32, tag="tmp")
                             nc.vector.tensor_mul(tmp, raw, cc_sb[:, soff : soff + TC])
                             nc.vector.tensor_mul(swp, swp, ss_sb[:, soff : soff + TC])
                             nc.vector.tensor_add(
                                 qk_sb[:, ot, soff : soff + TC], tmp, swp
                             )
                         for tt in range(TC // 128):
                             psv = ps1.tile([128, OC], f32, tag="ps_v")
                             for kk in range(KT):
                                 nc.tensor.matmul(
                                     psv,
                                     r(hch[kk][:, tt * 128 : (tt + 1) * 128]),
                                     r(wv_t[kk]),
                                     start=(kk == 0),
                                     stop=(kk == KT - 1),
                                 )
                             nc.scalar.copy(v_sb[:, tci * (TC // 128) + tt, :], psv)

                 # ---- phase 2: attention per head ----
                 outT_sb = qkvp.tile([128, HPC, S], fr, tag="outT")
                 with (
                     tc.tile_pool(name="ps2s", bufs=4, space="PSUM") as ps2s,
                     tc.tile_pool(name="ps2od", bufs=1, space="PSUM") as ps2od,
                 ):
                     for hl in range(HPC):
                         qTap = qk_sb[:, hl, :]
                         kTap = qk_sb[:, 2 + hl, :]
                         for qci in range(S // QC):
                             q0 = qci * QC
                             psO = ps2od.tile([128, QC], f32, tag="psO")
                             psD = ps2od.tile([128, QC], f32, tag="psD")
                             nkt = S // 128
                             for kg in range(nkt // 4):
                                 pexp = []
                                 for j in range(4):
                                     kt = kg * 4 + j
                                     pss = ps2s.tile([128, QC], f32, tag="pss")
                                     nc.tensor.matmul(
                                         pss,
                                         r(kTap[:, kt * 128 : (kt + 1) * 128]),
                                         r(qTap[:, q0 : q0 + QC]),
                                         skip_group_check=True,
                                     )
                                     pe = pp.tile([128, QC], fr, tag="pexp")
                                     nc.scalar.activation(pe, pss, Exp, scale=SCALE)
                                     pexp.append(pe)
                                 for j in range(4):
                                     kt = kg * 4 + j
                                     first = kt == 0
                                     last = kt == nkt - 1
                                     nc.tensor.matmul(
                                         psO,
                                         r(v_sb[:, kt, hl * DH : (hl + 1) * DH]),
                                         r(pexp[j]),
                                         start=first,
                                         stop=last,
                                         skip_group_check=True,
                                     )
                                     nc.tensor.matmul(
                                         psD,
                                         r(ones_sb),
                                         r(pexp[j]),
                                         start=first,
                                         stop=last,
                                         skip_group_check=True,
                                     )
                             rd = smallp.tile([128, QC], f32, tag="rd")
                             nc.vector.reciprocal(rd, psD)
                             nc.vector.tensor_mul(
                                 outT_sb[:, hl, q0 : q0 + QC], psO, rd
                             )

                 # ---- phase 3: output projection (partial over this core's heads) ----
                 with tc.tile_pool(name="ps3", bufs=4, space="PSUM") as ps3:
                     for tt in range(S // 128):
                         for nh in range(HID // 512):
                             psF = ps3.tile([128, 512], f32, tag="psF")
                             for hl in range(HPC):
                                 nc.tensor.matmul(
                                     psF,
                                     r(outT_sb[:, hl, tt * 128 : (tt + 1) * 128]),
                                     r(wo_t[hl][:, nh * 512 : (nh + 1) * 512]),
                                     start=(hl == 0),
                                     stop=(hl == HPC - 1),
                                 )
                             fo = foutp.tile([128, 512], f32, tag="fo")
                             nc.scalar.copy(fo, psF)
                             nc.sync.dma_start(
                                 out=out_p[
                                     t0 + tt * 128 : t0 + (tt + 1) * 128,
                                     nh * 512 : (nh + 1) * 512,
                                 ],
                                 in_=fo,
                             )

    nc.compile()
    return nc


def _deint(idx128):
    """de-interleave a [128] index block: evens then odds."""
    return np.concatenate([idx128[0::2], idx128[1::2]])


def _prep_inputs(hidden_states, cos, sin, w_qkv, w_o):
    """Host-side shard/layout prep. Returns per-core input maps."""
    hs = np.ascontiguousarray(
        hidden_states.reshape(T, HID).T, dtype=np.float32
    )  # [HID, T]
    ccf = np.ascontiguousarray(
        np.concatenate([cos.T[0::2, :], cos.T[1::2, :]], axis=0), dtype=np.float32
    )  # [128, S] de-interleaved
    ssf = np.ascontiguousarray(
        np.concatenate([-sin.T[0::2, :], sin.T[1::2, :]], axis=0), dtype=np.float32
    )  # [128, S] de-interleaved, sign folded

    in_maps = []
    for c in range(NC):
        heads = [HPC * c + i for i in range(HPC)]
        qrows = np.concatenate([_deint(np.arange(h * DH, (h + 1) * DH)) for h in heads])
        krows = H * DH + qrows
        vrows = (
            np.concatenate([np.arange(h * DH, (h + 1) * DH) for h in heads])
            + 2 * H * DH
        )
        ocols = np.concatenate([np.arange(h * DH, (h + 1) * DH) for h in heads])
        in_maps.append(
            {
                "hT": hs,
                "wqT": np.ascontiguousarray(w_qkv[qrows, :].T, dtype=np.float32),
                "wkT": np.ascontiguousarray(w_qkv[krows, :].T, dtype=np.float32),
                "wvT": np.ascontiguousarray(w_qkv[vrows, :].T, dtype=np.float32),
                "woT": np.ascontiguousarray(w_o[:, ocols].T, dtype=np.float32),
                "cc": ccf,
                "ss": ssf,
            }
        )
    return in_maps


def kernel(hidden_states, cos, sin, w_qkv, w_o):
    global _exec_time_ns
    from concourse.bass_utils import run_bass_kernel_spmd

    hidden_states = np.asarray(hidden_states, dtype=np.float32)
    cos = np.asarray(cos, dtype=np.float32)
    sin = np.asarray(sin, dtype=np.float32)
    w_qkv = np.asarray(w_qkv, dtype=np.float32)
    w_o = np.asarray(w_o, dtype=np.float32)

    nc = _build()
    in_maps = _prep_inputs(hidden_states, cos, sin, w_qkv, w_o)
    res = run_bass_kernel_spmd(
        nc,
        in_maps,
        core_ids=list(range(NC)),
        trace=bool(int(os.environ.get("KERNEL_TRACE", "0"))),
    )
    _exec_time_ns = res.exec_time_ns
    globals()["_last_result"] = res

    acc = res.results[0]["out_p"].astype(np.float32).copy()
    for c in range(1, NC):
        acc += res.results[c]["out_p"]
    return acc.reshape(B, S, HID)

